# revision 13
# baseline (speedup 1.0000x reference)
"""Causal self-attention Trainium2 Bass kernel (V5).

Full-input contract: kernel(**inputs) takes the unsharded inputs
(x [8,1024,768], W_attn [768,2304], b_attn [2304], W_proj [768,768],
b_proj [768]) and returns the full output [8,1024,768].

Sharding: data parallel - batch element b runs on NeuronCore b (B=8 =
n_cores), no collectives.

V5 structure (vs V4 @ 227us):
  - q^T/k^T m-tiles for head pair hp are computed at the START of pair
    hp's attention slot (sharing the "s" PSUM tag), so the PE stays
    dense through the whole attention phase and HAM never rethrottles
    (V4's phase C ran at 1.2 GHz: scores MMs measured 427ns vs 216
    warm).
  - softmax reciprocal is partition-major: the per-head l row [1,1024]
    is cast to bf16, DMA-scattered SBUF->SBUF to [8,128], reciprocal'd
    on 8 lanes (0.7us vs 6.5us free-dim-bound), and DMA-gathered back
    to a row for the ones-stationary broadcast matmul.
  - v bias is folded into b_proj on the host (y = sum p (v+b_v) =>
    y/l = (sum p v)/l + b_v, so b_eff = b_proj + b_v @ W_proj) -
    removes 2 matmuls per t-tile in the v phase.
  - proj runs from the same "s" PSUM tag right after the last pair,
    with the last head pair's c-tile (c=5) ordered last in each
    accumulation group so proj c0-c4 can start during the final
    divide chain.

Per-core layout:
  xT  [C, T] bf16   : DMA'd directly (host pre-transposed)
  qkT [2C, T] bf16  : q^T / k^T = W-tile.T-stationary @ xT + bias
  vA  [T, 65*H] bf16: v interleaved with a ones column per head so the
                      AV matmul also produces the softmax denominator l
  s^T [tk, tq] psum : k-slice.T @ q-slice per head; causality = only
                      columns tq >= 128i computed + one [128,128]
                      gpsimd affine_select on the diagonal sub-tile
  y^T [C, T] bf16   : (att @ v)^T accumulated in PSUM, evicted, then
                      multiplied by broadcast(1/l)
  out [T, C] f32    : y^T-tile.T-stationary @ W_proj + b_eff
"""

import os
import sys

import numpy as np

for _p in ("/opt/trn_rl_repo", "/root/.axon_site/_ro/trn_rl_repo"):
    if os.path.isdir(_p) and _p not in sys.path:
        sys.path.insert(0, _p)
        break

import concourse.bass as bass
import concourse.mybir as mybir
import concourse.tile as tile
from concourse.bass_utils import run_bass_kernel_spmd

T, C, H = 1024, 768, 12
C3 = 3 * C
NCORES = 8
NT = T // 128    # 8 t-tiles
NC_ = C // 128   # 6 c-tiles
NM = 2 * C // 128  # 12 m-tiles covering q,k output cols
f32 = mybir.dt.float32
bf16 = mybir.dt.bfloat16

EXP = mybir.ActivationFunctionType.Exp


def build_module():
    nc = bass.Bass()
    xT_d = nc.dram_tensor("xT", [C, T], bf16, kind="ExternalInput")
    wa_d = nc.dram_tensor("W_attn", [C, C3], bf16, kind="ExternalInput")
    ba_d = nc.dram_tensor("b_attn", [1, C3], f32, kind="ExternalInput")
    wp_d = nc.dram_tensor("W_proj", [C, C], bf16, kind="ExternalInput")
    bp_d = nc.dram_tensor("b_proj", [1, C], f32, kind="ExternalInput")
    out_d = nc.dram_tensor("out", [T, C], f32, kind="ExternalOutput")
    # DRAM bounce buffer for the l-row partition reshape (SBUF-side
    # partition reshapes are illegal DMA APs; DRAM is flat)
    ls_d = nc.dram_tensor("lscratch", [2 * H, T], bf16, kind="Internal")

    with tile.TileContext(nc) as tc:
        with tc.tile_pool(name="persist", bufs=1) as P0:
            ones_b = P0.tile([1, 128], bf16, name="ones_b")
            nc.vector.memset(ones_b[:], 1.0)
            # stationaries for the 1/l broadcast: rows 0 and 32 so the
            # lhsT base partition matches the gathered 1/l row
            ones33 = P0.tile([33, 64], bf16, name="ones33")
            nc.vector.memset(ones33[0:1, :], 1.0)
            nc.vector.memset(ones33[32:33, :], 1.0)

            xT = [P0.tile([128, T], bf16, name=f"xT{c}") for c in range(NC_)]
            qkT = [P0.tile([128, T], bf16, name=f"qkT{m}") for m in range(NM)]
            vA = [P0.tile([128, 65 * H], bf16, name=f"vA{t}") for t in range(NT)]
            yT = [P0.tile([128, T], bf16, name=f"yT{c}") for c in range(NC_)]
            wV = [P0.tile([128, C], bf16, name=f"wV{c}") for c in range(NC_)]
            wAq = [P0.tile([128, C], bf16, name=f"wAq{c}") for c in range(NC_)]
            wAk = [P0.tile([128, C], bf16, name=f"wAk{c}") for c in range(NC_)]
            wpt = [P0.tile([128, C], bf16, name=f"wp{c}") for c in range(NC_)]
            bqk = [P0.tile([128, 1], f32, name=f"bqk{m}") for m in range(NM)]
            bp_f = P0.tile([1, C], f32, name="bp_f")
            bp_sb = P0.tile([1, C], bf16, name="bp_sb")

            # ---- DMA emission order: x first, then weights ----
            for c in range(NC_):
                nc.sync.dma_start(out=xT[c][:], in_=xT_d[128 * c:128 * (c + 1), :])
            for c in range(NC_):
                nc.sync.dma_start(out=wV[c][:],
                                  in_=wa_d[128 * c:128 * (c + 1), 2 * C:3 * C])
            for c in range(NC_):
                nc.sync.dma_start(out=wAq[c][:],
                                  in_=wa_d[128 * c:128 * (c + 1), 0:C])
            for c in range(NC_):
                nc.sync.dma_start(out=wAk[c][:],
                                  in_=wa_d[128 * c:128 * (c + 1), C:2 * C])
            for m in range(NM):
                nc.sync.dma_start(
                    out=bqk[m][:],
                    in_=ba_d[0:1, 128 * m:128 * (m + 1)].rearrange("a p -> p a"))
            nc.sync.dma_start(out=bp_f[:], in_=bp_d[:])
            nc.vector.tensor_copy(bp_sb[:], bp_f[:])
            for c in range(NC_):
                nc.sync.dma_start(out=wpt[c][:], in_=wp_d[128 * c:128 * (c + 1), :])

            # ---- phase A: v (bias folded into b_proj on host) ----
            with tc.tile_pool(name="psA", bufs=1, space="PSUM") as PSA:
                for t in range(NT):
                    accv = PSA.tile([128, C], f32, tag="v", bufs=2, name="accv")
                    for c in range(NC_):
                        xcol = xT[c][:, 128 * t:128 * (t + 1)]
                        nc.tensor.matmul(accv[:, 0:512], xcol, wV[c][:, 0:512],
                                         start=(c == 0), stop=False)
                        nc.tensor.matmul(accv[:, 512:C], xcol, wV[c][:, 512:C],
                                         start=(c == 0), stop=(c == NC_ - 1))
                    av = vA[t].rearrange("p (h e) -> p h e", h=H)
                    nc.vector.memset(av[:, :, 64:65], 1.0)
                    nc.scalar.copy(av[:, :, 0:64],
                                   accv[:].rearrange("p (h e) -> p h e", h=H))

            # ---- merged qk + attention phase ----
            with tc.tile_pool(name="psC", bufs=1, space="PSUM") as PSC, \
                 tc.tile_pool(name="sbC", bufs=1) as SBC:
                pending = None  # (hp, [ySd_A, ySd_B], rlrow)

                def flush_divide(pend):
                    php, ySds, rlrow = pend
                    rlp = PSC.tile([128, T], f32, tag="s", bufs=2, name="rlp")
                    for hs in range(2):
                        b = 64 * hs
                        p = 32 * hs
                        nc.tensor.matmul(rlp[b:b + 64, 0:512],
                                         ones33[p:p + 1, 0:64],
                                         rlrow[p:p + 1, 0:512],
                                         start=True, stop=True)
                        nc.tensor.matmul(rlp[b:b + 64, 512:T],
                                         ones33[p:p + 1, 0:64],
                                         rlrow[p:p + 1, 512:T],
                                         start=True, stop=True)
                    for hs in range(2):
                        b = 64 * hs
                        nc.vector.tensor_mul(yT[php][b:b + 64, :],
                                             ySds[hs][:, :], rlp[b:b + 64, :])

                for hp in range(H // 2):
                    # q^T / k^T m-tiles for this pair (keeps PE dense)
                    for m in (hp, NC_ + hp):
                        wh = wAq if m < NC_ else wAk
                        mm = m % NC_
                        acc = PSC.tile([128, T], f32, tag="s", bufs=2,
                                       name="qkacc")
                        for c in range(NC_):
                            wa = wh[c][:, 128 * mm:128 * (mm + 1)]
                            for j2 in range(2):
                                nc.tensor.matmul(
                                    acc[:, 512 * j2:512 * (j2 + 1)],
                                    wa,
                                    xT[c][:, 512 * j2:512 * (j2 + 1)],
                                    start=(c == 0), stop=(c == NC_ - 1),
                                )
                        nc.vector.tensor_scalar_add(qkT[m][:], acc[:], bqk[m][:])

                    qt = qkT[hp]
                    kt = qkT[NC_ + hp]
                    avs = [PSC.tile([65, T], f32, tag="av", bufs=2, name="avp")
                           for _ in range(2)]
                    pbs = {}
                    for i in range(NT + 1):
                        if i < NT:
                            lo = 128 * i
                            sps = {}
                            for hs in range(2):
                                b = 64 * hs
                                sp = PSC.tile([128, T], f32, tag="s", bufs=2,
                                              name="sp")
                                ktile = kt[b:b + 64, 128 * i:128 * (i + 1)]
                                if lo < 512:
                                    nc.tensor.matmul(sp[:, lo:512], ktile,
                                                     qt[b:b + 64, lo:512],
                                                     start=True, stop=True)
                                    nc.tensor.matmul(sp[:, 512:T], ktile,
                                                     qt[b:b + 64, 512:T],
                                                     start=True, stop=True)
                                else:
                                    nc.tensor.matmul(sp[:, lo:T], ktile,
                                                     qt[b:b + 64, lo:T],
                                                     start=True, stop=True)
                                sps[hs] = sp
                            for hs in range(2):
                                pb = SBC.tile([128, T], bf16, tag="pb", bufs=4,
                                              name="pb")
                                nc.scalar.activation(pb[:, lo:T],
                                                     sps[hs][:, lo:T], EXP,
                                                     scale=0.125)
                                nc.gpsimd.affine_select(
                                    out=pb[:, lo:lo + 128],
                                    in_=pb[:, lo:lo + 128],
                                    pattern=[[1, 128]],
                                    compare_op=mybir.AluOpType.is_ge, fill=0.0,
                                    base=0, channel_multiplier=-1,
                                )
                                pbs[(i, hs)] = pb
                        if i == 4 and pending is not None:
                            flush_divide(pending)
                            pending = None
                        if i > 0:
                            ii = i - 1
                            lo = 128 * ii
                            for hs in range(2):
                                h = 2 * hp + hs
                                avp = avs[hs]
                                vt = vA[ii][:, 65 * h:65 * h + 65]
                                pb = pbs.pop((ii, hs))
                                if lo < 512:
                                    nc.tensor.matmul(avp[0:65, lo:512], vt,
                                                     pb[:, lo:512],
                                                     start=(ii == 0), stop=False,
                                                     skip_group_check=True)
                                    nc.tensor.matmul(avp[0:65, 512:T], vt,
                                                     pb[:, 512:T],
                                                     start=(ii == 0),
                                                     stop=(ii == NT - 1),
                                                     skip_group_check=True)
                                else:
                                    nc.tensor.matmul(avp[0:65, lo:T], vt,
                                                     pb[:, lo:T],
                                                     start=False,
                                                     stop=(ii == NT - 1),
                                                     skip_group_check=True)
                    # divide chain: l row -> bf16 -> scatter [8,128] ->
                    # reciprocal on 8 lanes -> gather back to a row
                    rl1 = SBC.tile([33, T], bf16, tag="rl1", bufs=2, name="rl1")
                    for hs in range(2):
                        nc.vector.tensor_copy(rl1[32 * hs:32 * hs + 1, :],
                                              avs[hs][64:65, :])
                    ySds = []
                    for hs in range(2):
                        ySd = SBC.tile([64, T], f32, tag="yS", bufs=4, name="yS")
                        nc.vector.tensor_copy(ySd[:], avs[hs][0:64, :])
                        ySds.append(ySd)
                    rin8 = SBC.tile([40, 128], bf16, tag="rin8", bufs=2,
                                    name="rin8")
                    rl8 = SBC.tile([40, 128], bf16, tag="rl8", bufs=2,
                                   name="rl8")
                    for hs in range(2):
                        h = 2 * hp + hs
                        nc.sync.dma_start(out=ls_d[h:h + 1, :],
                                          in_=rl1[32 * hs:32 * hs + 1, :])
                        nc.sync.dma_start(
                            out=rin8[32 * hs:32 * hs + 8, 0:128],
                            in_=ls_d[h:h + 1, :].rearrange(
                                "a (b c) -> (a b) c", b=8))
                    with nc.allow_low_precision(reason="1/l in bf16"):
                        nc.vector.reciprocal(rl8[0:40, :], rin8[0:40, :])
                    rlrow = SBC.tile([33, T], bf16, tag="rlrow", bufs=2,
                                     name="rlrow")
                    for hs in range(2):
                        h = H + 2 * hp + hs
                        nc.sync.dma_start(out=ls_d[h:h + 1, :].rearrange(
                                              "a (b c) -> (a b) c", b=8),
                                          in_=rl8[32 * hs:32 * hs + 8, 0:128])
                        nc.sync.dma_start(out=rlrow[32 * hs:32 * hs + 1, :],
                                          in_=ls_d[h:h + 1, :])
                    pending = (hp, ySds, rlrow)

                # ---- proj: out = y^T.T @ W_proj + b_eff ----
                # c in 0..4 first so proj overlaps the final divide chain
                for t in range(NT):
                    acc = PSC.tile([128, T], f32, tag="s", bufs=2, name="pjacc")
                    for c in range(NC_ - 1):
                        ycol = yT[c][:, 128 * t:128 * (t + 1)]
                        nc.tensor.matmul(acc[:, 0:512], ycol, wpt[c][:, 0:512],
                                         start=(c == 0), stop=False)
                        nc.tensor.matmul(acc[:, 512:C], ycol, wpt[c][:, 512:C],
                                         start=(c == 0), stop=False)
                    if t == 0:
                        flush_divide(pending)
                        pending = None
                    c = NC_ - 1
                    ycol = yT[c][:, 128 * t:128 * (t + 1)]
                    nc.tensor.matmul(acc[:, 0:512], ycol, wpt[c][:, 0:512],
                                     start=False, stop=False)
                    nc.tensor.matmul(acc[:, 512:C], ycol, wpt[c][:, 512:C],
                                     start=False, stop=False)
                    nc.tensor.matmul(acc[:, 0:512], ones_b[:],
                                     bp_sb[0:1, 0:512], start=False, stop=True)
                    nc.tensor.matmul(acc[:, 512:C], ones_b[:],
                                     bp_sb[0:1, 512:C], start=False, stop=True)
                    ot = SBC.tile([128, C], f32, tag="ot", bufs=3, name="ot")
                    nc.scalar.copy(ot[:], acc[:, 0:C])
                    nc.sync.dma_start(out=out_d[128 * t:128 * (t + 1), :],
                                      in_=ot[:])

    return nc


_WAIT_SKIP = {"InstNoOp", "InstEventSemOp", "InstSemaphoreOp"}


def _legalize_waits(nc):
    """walrus's codegen allows limited sync-wait commands per ISA struct
    (e.g. a Matmult's waits all land on the generated LDWEIGHTS struct which
    has one slot). Move excess waits onto same-engine NoOps inserted
    immediately before the instruction - program order on the engine queue
    preserves the synchronization semantics."""
    nfix = 0
    for fn in nc.m.functions:
        for bb in fn.blocks:
            out = []
            for ins in bb.instructions:
                si = ins.sync_info
                if (type(ins).__name__ not in _WAIT_SKIP and si is not None
                        and si.on_wait and len(si.on_wait) > 1):
                    waits = list(si.on_wait)
                    extra, keep = waits[:-1], waits[-1:]
                    for k, w in enumerate(extra):
                        nop = mybir.InstNoOp(name=f"{ins.name}-wf{k}", ins=[], outs=[])
                        nop.engine = ins.engine
                        nop.sync_info = mybir.SyncInfo(on_wait=[w], on_update=[])
                        out.append(nop)
                    ins.sync_info = mybir.SyncInfo(
                        on_wait=keep, on_update=list(si.on_update or []))
                    nfix += 1
                out.append(ins)
            bb.instructions = out
    return nfix


_cached_module = None


def _get_module():
    global _cached_module
    if _cached_module is None:
        nc = build_module()
        _legalize_waits(nc)
        _cached_module = nc
    return _cached_module


def make_in_maps(x, W_attn, b_attn, W_proj, b_proj):
    import ml_dtypes

    bf = ml_dtypes.bfloat16
    x = np.asarray(x, dtype=np.float32)
    wa_f = np.asarray(W_attn, dtype=np.float32)
    ba_f = np.asarray(b_attn, dtype=np.float32).reshape(C3)
    wp_f = np.asarray(W_proj, dtype=np.float32)
    bp_f = np.asarray(b_proj, dtype=np.float32).reshape(C)
    # v bias folds through the attention average into a constant shift of
    # y, so it lands in the proj bias: b_eff = b_proj + b_v @ W_proj
    bp_eff = bp_f + ba_f[2 * C:] @ wp_f
    wa = np.ascontiguousarray(wa_f.astype(bf))
    ba = np.ascontiguousarray(ba_f.reshape(1, C3))
    wp = np.ascontiguousarray(wp_f.astype(bf))
    bp = np.ascontiguousarray(bp_eff.reshape(1, C))
    return [
        dict(xT=np.ascontiguousarray(x[b].T.astype(bf)), W_attn=wa, b_attn=ba,
             W_proj=wp, b_proj=bp)
        for b in range(x.shape[0])
    ]


def run(x, W_attn, b_attn, W_proj, b_proj, trace=False, **spmd_kwargs):
    nc = _get_module()
    in_maps = make_in_maps(x, W_attn, b_attn, W_proj, b_proj)
    res = run_bass_kernel_spmd(nc, in_maps, list(range(NCORES)), trace=trace,
                               **spmd_kwargs)
    out = np.stack([res.results[b]["out"] for b in range(len(in_maps))], axis=0)
    return out, res


def kernel(x, W_attn, b_attn, W_proj, b_proj):
    out, _ = run(x, W_attn, b_attn, W_proj, b_proj)
    return out


# revision 16
# speedup vs baseline: 1.1732x; 1.1732x over previous
"""Causal self-attention Trainium2 Bass kernel (V5).

Full-input contract: kernel(**inputs) takes the unsharded inputs
(x [8,1024,768], W_attn [768,2304], b_attn [2304], W_proj [768,768],
b_proj [768]) and returns the full output [8,1024,768].

Sharding: data parallel - batch element b runs on NeuronCore b (B=8 =
n_cores), no collectives.

V5 structure (vs V4 @ 227us):
  - q^T/k^T m-tiles for head pair hp are computed at the START of pair
    hp's attention slot (sharing the "s" PSUM tag), so the PE stays
    dense through the whole attention phase and HAM never rethrottles
    (V4's phase C ran at 1.2 GHz: scores MMs measured 427ns vs 216
    warm).
  - softmax reciprocal is partition-major: the per-head l row [1,1024]
    is cast to bf16, DMA-scattered SBUF->SBUF to [8,128], reciprocal'd
    on 8 lanes (0.7us vs 6.5us free-dim-bound), and DMA-gathered back
    to a row for the ones-stationary broadcast matmul.
  - v bias is folded into b_proj on the host (y = sum p (v+b_v) =>
    y/l = (sum p v)/l + b_v, so b_eff = b_proj + b_v @ W_proj) -
    removes 2 matmuls per t-tile in the v phase.
  - proj runs from the same "s" PSUM tag right after the last pair,
    with the last head pair's c-tile (c=5) ordered last in each
    accumulation group so proj c0-c4 can start during the final
    divide chain.

Per-core layout:
  xT  [C, T] bf16   : DMA'd directly (host pre-transposed)
  qkT [2C, T] bf16  : q^T / k^T = W-tile.T-stationary @ xT + bias
  vA  [T, 65*H] bf16: v interleaved with a ones column per head so the
                      AV matmul also produces the softmax denominator l
  s^T [tk, tq] psum : k-slice.T @ q-slice per head; causality = only
                      columns tq >= 128i computed + one [128,128]
                      gpsimd affine_select on the diagonal sub-tile
  y^T [C, T] bf16   : (att @ v)^T accumulated in PSUM, evicted, then
                      multiplied by broadcast(1/l)
  out [T, C] f32    : y^T-tile.T-stationary @ W_proj + b_eff
"""

import os
import sys

import numpy as np

for _p in ("/opt/trn_rl_repo", "/root/.axon_site/_ro/trn_rl_repo"):
    if os.path.isdir(_p) and _p not in sys.path:
        sys.path.insert(0, _p)
        break

import concourse.bass as bass
import concourse.mybir as mybir
import concourse.tile as tile
from concourse.bass_utils import run_bass_kernel_spmd

T, C, H = 1024, 768, 12
C3 = 3 * C
NCORES = 8
NT = T // 128    # 8 t-tiles
NC_ = C // 128   # 6 c-tiles
NM = 2 * C // 128  # 12 m-tiles covering q,k output cols
f32 = mybir.dt.float32
bf16 = mybir.dt.bfloat16

EXP = mybir.ActivationFunctionType.Exp


def build_module():
    nc = bass.Bass()
    xT_d = nc.dram_tensor("xT", [C, T], bf16, kind="ExternalInput")
    wa_d = nc.dram_tensor("W_attn", [C, C3], bf16, kind="ExternalInput")
    ba_d = nc.dram_tensor("b_attn", [1, C3], f32, kind="ExternalInput")
    wp_d = nc.dram_tensor("W_proj", [C, C], bf16, kind="ExternalInput")
    bp_d = nc.dram_tensor("b_proj", [1, C], f32, kind="ExternalInput")
    out_d = nc.dram_tensor("out", [T, C], f32, kind="ExternalOutput")
    # DRAM bounce buffer for the l-row partition reshape (SBUF-side
    # partition reshapes are illegal DMA APs; DRAM is flat)
    ls_d = nc.dram_tensor("lscratch", [2 * H, T], bf16, kind="Internal")

    with tile.TileContext(nc) as tc:
        with tc.tile_pool(name="persist", bufs=1) as P0:
            ones_b = P0.tile([1, 128], bf16, name="ones_b")
            nc.vector.memset(ones_b[:], 1.0)
            # stationaries for the 1/l broadcast: rows 0 and 32 so the
            # lhsT base partition matches the gathered 1/l row
            ones33 = P0.tile([33, 64], bf16, name="ones33")
            nc.vector.memset(ones33[0:1, :], 1.0)
            nc.vector.memset(ones33[32:33, :], 1.0)

            xT = [P0.tile([128, T], bf16, name=f"xT{c}") for c in range(NC_)]
            qkT = [P0.tile([128, T], bf16, name=f"qkT{m}") for m in range(NC_)]
            # per-head k^T tiles, zero-padded to full 128 partitions: the
            # scores stationary is then [128,128] (full PE array) instead
            # of [64,128] - same streamed columns, but the PE activity
            # monitor sees full-row matmuls and keeps the clock unthrottled
            kz = [P0.tile([128, T], bf16, name=f"kz{h}") for h in range(H)]
            for h in range(H):
                z = 64 * (1 - (h % 2))
                nc.vector.memset(kz[h][z:z + 64, :], 0.0)
            vA = [P0.tile([128, 65 * H], bf16, name=f"vA{t}") for t in range(NT)]
            yT = [P0.tile([128, T], bf16, name=f"yT{c}") for c in range(NC_)]
            wV = [P0.tile([128, C], bf16, name=f"wV{c}") for c in range(NC_)]
            wAq = [P0.tile([128, C], bf16, name=f"wAq{c}") for c in range(NC_)]
            wAk = [P0.tile([128, C], bf16, name=f"wAk{c}") for c in range(NC_)]
            wpt = [P0.tile([128, C], bf16, name=f"wp{c}") for c in range(NC_)]
            bqk = [P0.tile([128, 1], f32, name=f"bqk{m}") for m in range(NM)]
            bp_f = P0.tile([1, C], f32, name="bp_f")
            bp_sb = P0.tile([1, C], bf16, name="bp_sb")

            # ---- DMA emission order: x first, then weights ----
            for c in range(NC_):
                nc.sync.dma_start(out=xT[c][:], in_=xT_d[128 * c:128 * (c + 1), :])
            for c in range(NC_):
                nc.sync.dma_start(out=wV[c][:],
                                  in_=wa_d[128 * c:128 * (c + 1), 2 * C:3 * C])
            for c in range(NC_):
                nc.sync.dma_start(out=wAq[c][:],
                                  in_=wa_d[128 * c:128 * (c + 1), 0:C])
            for c in range(NC_):
                nc.sync.dma_start(out=wAk[c][:],
                                  in_=wa_d[128 * c:128 * (c + 1), C:2 * C])
            for m in range(NM):
                nc.sync.dma_start(
                    out=bqk[m][:],
                    in_=ba_d[0:1, 128 * m:128 * (m + 1)].rearrange("a p -> p a"))
            nc.sync.dma_start(out=bp_f[:], in_=bp_d[:])
            nc.vector.tensor_copy(bp_sb[:], bp_f[:])
            for c in range(NC_):
                nc.sync.dma_start(out=wpt[c][:], in_=wp_d[128 * c:128 * (c + 1), :])

            # ---- phase A: v (bias folded into b_proj on host) ----
            with tc.tile_pool(name="psA", bufs=1, space="PSUM") as PSA:
                for t in range(NT):
                    accv = PSA.tile([128, C], f32, tag="v", bufs=2, name="accv")
                    for c in range(NC_):
                        xcol = xT[c][:, 128 * t:128 * (t + 1)]
                        nc.tensor.matmul(accv[:, 0:512], xcol, wV[c][:, 0:512],
                                         start=(c == 0), stop=False)
                        nc.tensor.matmul(accv[:, 512:C], xcol, wV[c][:, 512:C],
                                         start=(c == 0), stop=(c == NC_ - 1))
                    av = vA[t].rearrange("p (h e) -> p h e", h=H)
                    nc.vector.memset(av[:, :, 64:65], 1.0)
                    nc.scalar.copy(av[:, :, 0:64],
                                   accv[:].rearrange("p (h e) -> p h e", h=H))

            # ---- merged qk + attention phase ----
            with tc.tile_pool(name="psC", bufs=1, space="PSUM") as PSC, \
                 tc.tile_pool(name="sbC", bufs=1) as SBC:
                pending = None  # (hp, [ySd_A, ySd_B], rlrow)

                def flush_divide(pend):
                    php, ySds, rlrow = pend
                    rlp = PSC.tile([128, T], f32, tag="s", bufs=2, name="rlp")
                    for hs in range(2):
                        b = 64 * hs
                        p = 32 * hs
                        nc.tensor.matmul(rlp[b:b + 64, 0:512],
                                         ones33[p:p + 1, 0:64],
                                         rlrow[p:p + 1, 0:512],
                                         start=True, stop=True)
                        nc.tensor.matmul(rlp[b:b + 64, 512:T],
                                         ones33[p:p + 1, 0:64],
                                         rlrow[p:p + 1, 512:T],
                                         start=True, stop=True)
                    for hs in range(2):
                        b = 64 * hs
                        nc.vector.tensor_mul(yT[php][b:b + 64, :],
                                             ySds[hs][:, :], rlp[b:b + 64, :])

                for hp in range(H // 2):
                    # q^T / k^T m-tiles for this pair (keeps PE dense)
                    for m in (hp, NC_ + hp):
                        wh = wAq if m < NC_ else wAk
                        mm = m % NC_
                        acc = PSC.tile([128, T], f32, tag="s", bufs=2,
                                       name="qkacc")
                        for c in range(NC_):
                            wa = wh[c][:, 128 * mm:128 * (mm + 1)]
                            for j2 in range(2):
                                nc.tensor.matmul(
                                    acc[:, 512 * j2:512 * (j2 + 1)],
                                    wa,
                                    xT[c][:, 512 * j2:512 * (j2 + 1)],
                                    start=(c == 0), stop=(c == NC_ - 1),
                                )
                        if m < NC_:
                            nc.vector.tensor_scalar_add(qkT[m][:], acc[:],
                                                        bqk[m][:])
                        else:
                            for hs in range(2):
                                b = 64 * hs
                                nc.vector.tensor_scalar_add(
                                    kz[2 * hp + hs][b:b + 64, :],
                                    acc[b:b + 64, :], bqk[m][b:b + 64, :])

                    qt = qkT[hp]
                    avs = [PSC.tile([65, T], f32, tag="av", bufs=2, name="avp")
                           for _ in range(2)]
                    pbs = {}
                    for i in range(NT + 1):
                        if i < NT:
                            lo = 128 * i
                            sps = {}
                            for hs in range(2):
                                sp = PSC.tile([128, T], f32, tag="s", bufs=2,
                                              name="sp")
                                ktile = kz[2 * hp + hs][:,
                                                        128 * i:128 * (i + 1)]
                                if lo < 512:
                                    nc.tensor.matmul(sp[:, lo:512], ktile,
                                                     qt[:, lo:512],
                                                     start=True, stop=True)
                                    nc.tensor.matmul(sp[:, 512:T], ktile,
                                                     qt[:, 512:T],
                                                     start=True, stop=True)
                                else:
                                    nc.tensor.matmul(sp[:, lo:T], ktile,
                                                     qt[:, lo:T],
                                                     start=True, stop=True)
                                sps[hs] = sp
                            for hs in range(2):
                                pb = SBC.tile([128, T], bf16, tag="pb", bufs=4,
                                              name="pb")
                                nc.scalar.activation(pb[:, lo:T],
                                                     sps[hs][:, lo:T], EXP,
                                                     scale=0.125)
                                nc.gpsimd.affine_select(
                                    out=pb[:, lo:lo + 128],
                                    in_=pb[:, lo:lo + 128],
                                    pattern=[[1, 128]],
                                    compare_op=mybir.AluOpType.is_ge, fill=0.0,
                                    base=0, channel_multiplier=-1,
                                )
                                pbs[(i, hs)] = pb
                        if i == 4 and pending is not None:
                            flush_divide(pending)
                            pending = None
                        if i > 0:
                            ii = i - 1
                            lo = 128 * ii
                            for hs in range(2):
                                h = 2 * hp + hs
                                avp = avs[hs]
                                vt = vA[ii][:, 65 * h:65 * h + 65]
                                pb = pbs.pop((ii, hs))
                                if lo < 512:
                                    nc.tensor.matmul(avp[0:65, lo:512], vt,
                                                     pb[:, lo:512],
                                                     start=(ii == 0), stop=False,
                                                     skip_group_check=True)
                                    nc.tensor.matmul(avp[0:65, 512:T], vt,
                                                     pb[:, 512:T],
                                                     start=(ii == 0),
                                                     stop=(ii == NT - 1),
                                                     skip_group_check=True)
                                else:
                                    nc.tensor.matmul(avp[0:65, lo:T], vt,
                                                     pb[:, lo:T],
                                                     start=False,
                                                     stop=(ii == NT - 1),
                                                     skip_group_check=True)
                    # divide chain: l row -> bf16 -> scatter [8,128] ->
                    # reciprocal on 8 lanes -> gather back to a row
                    rl1 = SBC.tile([33, T], bf16, tag="rl1", bufs=2, name="rl1")
                    for hs in range(2):
                        nc.vector.tensor_copy(rl1[32 * hs:32 * hs + 1, :],
                                              avs[hs][64:65, :])
                    ySds = []
                    for hs in range(2):
                        ySd = SBC.tile([64, T], f32, tag="yS", bufs=4, name="yS")
                        nc.vector.tensor_copy(ySd[:], avs[hs][0:64, :])
                        ySds.append(ySd)
                    rin8 = SBC.tile([40, 128], bf16, tag="rin8", bufs=2,
                                    name="rin8")
                    rl8 = SBC.tile([40, 128], bf16, tag="rl8", bufs=2,
                                   name="rl8")
                    for hs in range(2):
                        h = 2 * hp + hs
                        nc.sync.dma_start(out=ls_d[h:h + 1, :],
                                          in_=rl1[32 * hs:32 * hs + 1, :])
                        nc.sync.dma_start(
                            out=rin8[32 * hs:32 * hs + 8, 0:128],
                            in_=ls_d[h:h + 1, :].rearrange(
                                "a (b c) -> (a b) c", b=8))
                    with nc.allow_low_precision(reason="1/l in bf16"):
                        nc.vector.reciprocal(rl8[0:40, :], rin8[0:40, :])
                    rlrow = SBC.tile([33, T], bf16, tag="rlrow", bufs=2,
                                     name="rlrow")
                    for hs in range(2):
                        h = H + 2 * hp + hs
                        nc.sync.dma_start(out=ls_d[h:h + 1, :].rearrange(
                                              "a (b c) -> (a b) c", b=8),
                                          in_=rl8[32 * hs:32 * hs + 8, 0:128])
                        nc.sync.dma_start(out=rlrow[32 * hs:32 * hs + 1, :],
                                          in_=ls_d[h:h + 1, :])
                    pending = (hp, ySds, rlrow)

                # ---- proj: out = y^T.T @ W_proj + b_eff ----
                # c in 0..4 first so proj overlaps the final divide chain
                for t in range(NT):
                    acc = PSC.tile([128, T], f32, tag="s", bufs=2, name="pjacc")
                    for c in range(NC_ - 1):
                        ycol = yT[c][:, 128 * t:128 * (t + 1)]
                        nc.tensor.matmul(acc[:, 0:512], ycol, wpt[c][:, 0:512],
                                         start=(c == 0), stop=False)
                        nc.tensor.matmul(acc[:, 512:C], ycol, wpt[c][:, 512:C],
                                         start=(c == 0), stop=False)
                    if t == 0:
                        flush_divide(pending)
                        pending = None
                    c = NC_ - 1
                    ycol = yT[c][:, 128 * t:128 * (t + 1)]
                    nc.tensor.matmul(acc[:, 0:512], ycol, wpt[c][:, 0:512],
                                     start=False, stop=False)
                    nc.tensor.matmul(acc[:, 512:C], ycol, wpt[c][:, 512:C],
                                     start=False, stop=False)
                    nc.tensor.matmul(acc[:, 0:512], ones_b[:],
                                     bp_sb[0:1, 0:512], start=False, stop=True)
                    nc.tensor.matmul(acc[:, 512:C], ones_b[:],
                                     bp_sb[0:1, 512:C], start=False, stop=True)
                    ot = SBC.tile([128, C], f32, tag="ot", bufs=3, name="ot")
                    nc.scalar.copy(ot[:], acc[:, 0:C])
                    nc.sync.dma_start(out=out_d[128 * t:128 * (t + 1), :],
                                      in_=ot[:])

    return nc


_WAIT_SKIP = {"InstNoOp", "InstEventSemOp", "InstSemaphoreOp"}


def _legalize_waits(nc):
    """walrus's codegen allows limited sync-wait commands per ISA struct
    (e.g. a Matmult's waits all land on the generated LDWEIGHTS struct which
    has one slot). Move excess waits onto same-engine NoOps inserted
    immediately before the instruction - program order on the engine queue
    preserves the synchronization semantics."""
    nfix = 0
    for fn in nc.m.functions:
        for bb in fn.blocks:
            out = []
            for ins in bb.instructions:
                si = ins.sync_info
                if (type(ins).__name__ not in _WAIT_SKIP and si is not None
                        and si.on_wait and len(si.on_wait) > 1):
                    waits = list(si.on_wait)
                    extra, keep = waits[:-1], waits[-1:]
                    for k, w in enumerate(extra):
                        nop = mybir.InstNoOp(name=f"{ins.name}-wf{k}", ins=[], outs=[])
                        nop.engine = ins.engine
                        nop.sync_info = mybir.SyncInfo(on_wait=[w], on_update=[])
                        out.append(nop)
                    ins.sync_info = mybir.SyncInfo(
                        on_wait=keep, on_update=list(si.on_update or []))
                    nfix += 1
                out.append(ins)
            bb.instructions = out
    return nfix


_cached_module = None


def _get_module():
    global _cached_module
    if _cached_module is None:
        nc = build_module()
        _legalize_waits(nc)
        _cached_module = nc
    return _cached_module


def make_in_maps(x, W_attn, b_attn, W_proj, b_proj):
    import ml_dtypes

    bf = ml_dtypes.bfloat16
    x = np.asarray(x, dtype=np.float32)
    wa_f = np.asarray(W_attn, dtype=np.float32)
    ba_f = np.asarray(b_attn, dtype=np.float32).reshape(C3)
    wp_f = np.asarray(W_proj, dtype=np.float32)
    bp_f = np.asarray(b_proj, dtype=np.float32).reshape(C)
    # v bias folds through the attention average into a constant shift of
    # y, so it lands in the proj bias: b_eff = b_proj + b_v @ W_proj
    bp_eff = bp_f + ba_f[2 * C:] @ wp_f
    wa = np.ascontiguousarray(wa_f.astype(bf))
    ba = np.ascontiguousarray(ba_f.reshape(1, C3))
    wp = np.ascontiguousarray(wp_f.astype(bf))
    bp = np.ascontiguousarray(bp_eff.reshape(1, C))
    return [
        dict(xT=np.ascontiguousarray(x[b].T.astype(bf)), W_attn=wa, b_attn=ba,
             W_proj=wp, b_proj=bp)
        for b in range(x.shape[0])
    ]


def run(x, W_attn, b_attn, W_proj, b_proj, trace=False, **spmd_kwargs):
    nc = _get_module()
    in_maps = make_in_maps(x, W_attn, b_attn, W_proj, b_proj)
    res = run_bass_kernel_spmd(nc, in_maps, list(range(NCORES)), trace=trace,
                               **spmd_kwargs)
    out = np.stack([res.results[b]["out"] for b in range(len(in_maps))], axis=0)
    return out, res


def kernel(x, W_attn, b_attn, W_proj, b_proj):
    out, _ = run(x, W_attn, b_attn, W_proj, b_proj)
    return out
